# revision 22
# baseline (speedup 1.0000x reference)
"""Distributed Trainium2 kernel for the AEN (attentive episodic network) problem.

Reference computation (shapes):
    support_vs = support @ Wv.T + bv                    [8192, 512]
    q_proto    = queries @ Wv.T + bv                    [8192, 512]
    support_ks = LN(support @ Wk.T + bk)                [8192, 512]
    queries_qs = LN(queries @ Wq.T + bq)                [8192, 512]
    scores     = queries_qs @ support_ks.T / sqrt(512)  [8192, 8192]
    affinity   = softmax(scores, axis=1)
    class_proto= affinity @ support_vs                  [8192, 512]
    returns (q_proto, class_proto)

Sharding: queries split 1024/core across the 8 cores; support set and weights
replicated — every core computes the full support K/V projections locally.
(Collectives on this runtime have a ~2.5 ms latency floor, measured, so +2x
redundant projection FLOPs beat any AllGather by ~5x.)

Runtime cost model (measured on this fleet):
  - PE matmul ~151 ns for [128x128]@[128x512] bf16; PE is cheap.
  - ScalarE (ACT) ops reading f32 cost ~80 us EACH regardless of size; ops
    reading bf16/fp16 are normal.  So exp() reads an fp16 staging copy of the
    scores, never PSUM f32 directly, and all other ACT ops have 16-bit input.
  - DVE ops ~1-6 us each (cheaper with 16-bit output); keep the op count low:
    projection biases are folded into the matmul accumulation as a K=1
    [ones-column x bias-row] matmul instead of DVE adds.

On-chip layouts: activations arrive feature-major ([d, tok] in 128x128
blocks, host pre-packed) so projections emit token-major tiles directly;
normalized K / queries are PE-transposed per 128x128 block into feature-major
for the scores matmul (LN gamma/beta fold into the post-transpose PSUM->SBUF
copy on ACT, which is free for bf16 input).  exp(scores.T) tiles [s, q] then
serve directly as lhsT for both attention@V (token-major out, no transposes
of the 8192x1024 probability matrix) and the softmax denominator (rhs = ones
column).  All matmuls bf16 with f32 PSUM accumulation.
"""

import os

import ml_dtypes
import numpy as np

D = 1024  # model dim
O = 512  # out dim
NCORES = 8
NL = 1024  # query rows per core
NS = 8192  # support rows (replicated)
NMT = NL // 128  # 8 query token tiles per core
NST = NS // 128  # 64 support token tiles
NDT = D // 128  # 8 contraction tiles
NOT = O // 128  # 4 outdim tiles
SCALE = 1.0 / float(np.sqrt(np.float32(O)))
LN_EPS = 1e-5
BF16 = ml_dtypes.bfloat16

_CACHE: dict = {}

LAST_RESULTS = None


def _build_graph(reps=1, main_blocks=None):
    import concourse.bass as bass  # noqa: F401
    import concourse.tile as tile
    from concourse import bacc, mybir
    from concourse.masks import make_identity

    f32 = mybir.dt.float32
    f16 = mybir.dt.float16
    bf16 = mybir.dt.bfloat16
    Alu = mybir.AluOpType
    Act = mybir.ActivationFunctionType

    n_st = NST if main_blocks is None else main_blocks * NMT

    nc = bacc.Bacc(
        "TRN2", target_bir_lowering=False, debug=False, num_devices=NCORES
    )

    sTp = nc.dram_tensor("sTp", [NST, 128, D], bf16, kind="ExternalInput").ap()
    qTp = nc.dram_tensor("qTp", [NMT, 128, D], bf16, kind="ExternalInput").ap()
    w = nc.dram_tensor("w", [D, 3 * O], bf16, kind="ExternalInput").ap()
    # bias rows [1, 1536] = [bq | bk | bv], applied via K=1 matmul
    brow = nc.dram_tensor("brow", [1, 3 * O], bf16, kind="ExternalInput").ap()
    g_p = nc.dram_tensor("g_p", [O, 1], f32, kind="ExternalInput").ap()
    be_p = nc.dram_tensor("be_p", [O, 1], f32, kind="ExternalInput").ap()
    out_q = nc.dram_tensor("out_q", [NL, O], f32, kind="ExternalOutput").ap()
    out_c = nc.dram_tensor("out_c", [NL, O], f32, kind="ExternalOutput").ap()

    from contextlib import ExitStack

    with tile.TileContext(nc) as tc:
        with ExitStack() as ctx:
            ent = ctx.enter_context
            consts = ent(tc.tile_pool(name="consts", bufs=1))
            wp = ent(tc.tile_pool(name="wp", bufs=NDT))
            sp = ent(tc.tile_pool(name="sp", bufs=3))
            stp = ent(tc.tile_pool(name="stp", bufs=10))
            yp = ent(tc.tile_pool(name="yp", bufs=3))
            ktp = ent(tc.tile_pool(name="ktp", bufs=1))
            qqp = ent(tc.tile_pool(name="qqp", bufs=NOT))
            vap = ent(tc.tile_pool(name="vap", bufs=1))
            scp = ent(tc.tile_pool(name="scp", bufs=4))
            exl = ent(tc.tile_pool(name="exl", bufs=6))
            ocp = ent(tc.tile_pool(name="ocp", bufs=3))
            psA = ent(tc.tile_pool(name="psA", bufs=4, space="PSUM"))
            psB = ent(tc.tile_pool(name="psB", bufs=3, space="PSUM"))
            psS = ent(tc.tile_pool(name="psS", bufs=1, space="PSUM"))

            ident = consts.tile([128, 128], bf16, name="ident")
            make_identity(nc, ident)
            ones = consts.tile([128, 1], bf16, name="ones")
            nc.vector.memset(ones, 1.0)
            ones_row = consts.tile([1, 128], bf16, name="ones_row")
            nc.vector.memset(ones_row, 1.0)
            eps_t = consts.tile([128, 1], f32, name="eps_t")
            nc.vector.memset(eps_t, LN_EPS)
            brow_sb = consts.tile([1, 3 * O], bf16, name="brow_sb")
            nc.sync.dma_start(out=brow_sb, in_=brow)
            gam = []
            bet = []
            for j in range(NOT):
                g_t = consts.tile([128, 1], f32, name=f"g{j}")
                nc.sync.dma_start(out=g_t, in_=g_p[j * 128 : (j + 1) * 128, :])
                gam.append(g_t)
                b_t = consts.tile([128, 1], f32, name=f"b{j}")
                nc.sync.dma_start(out=b_t, in_=be_p[j * 128 : (j + 1) * 128, :])
                bet.append(b_t)

            wt = []
            for k in range(NDT):
                wtk = wp.tile([128, 3 * O], bf16, name=f"wt{k}", tag="wt")
                nc.sync.dma_start(out=wtk, in_=w[k * 128 : (k + 1) * 128, :])
                wt.append(wtk)

            for _rep in range(reps):
                # K feature-major [o, s] (4 x 8192 sections) + V token-major,
                # both SBUF-resident for the whole attention loop
                kT = ktp.tile([128, NOT * NS], bf16, name="kT", tag="ktp")
                v_all = vap.tile([128, NST * O], bf16, name="v_all", tag="vap")
                qqT = [
                    qqp.tile([128, NL], bf16, name=f"qqT{j}", tag="qq")
                    for j in range(NOT)
                ]

                def proj_pair(xt, off_a, off_b):
                    # two projections sharing the lhsT activation tile, each
                    # with the bias row folded in as a K=1 matmul
                    ps_a = psA.tile([128, O], f32, name="ps_a", tag="psA")
                    ps_b = psA.tile([128, O], f32, name="ps_b", tag="psA")
                    for k in range(NDT):
                        lhs = xt[:, k * 128 : (k + 1) * 128]
                        nc.tensor.matmul(
                            ps_a, lhs, wt[k][:, off_a : off_a + O],
                            start=(k == 0), stop=False,
                        )
                        nc.tensor.matmul(
                            ps_b, lhs, wt[k][:, off_b : off_b + O],
                            start=(k == 0), stop=False,
                        )
                    nc.tensor.matmul(
                        ps_a, ones_row, brow_sb[:, off_a : off_a + O],
                        start=False, stop=True,
                    )
                    nc.tensor.matmul(
                        ps_b, ones_row, brow_sb[:, off_b : off_b + O],
                        start=False, stop=True,
                    )
                    return ps_a, ps_b

                def ln_transpose(ps, dst_fn, m):
                    # LN stats straight off PSUM; normalize to bf16; PE-
                    # transpose each 128-block; gamma/beta fused into the
                    # (bf16-input, hence fast) ACT copy out of PSUM
                    stats = stp.tile([128, 6], f32, name="stats", tag="stp")
                    nc.vector.bn_stats(stats, ps)
                    mv = stp.tile([128, 2], f32, name="mv", tag="stp")
                    nc.vector.bn_aggr(mv, stats)
                    rstd = stp.tile([128, 1], f32, name="rstd", tag="stp")
                    nc.scalar.activation(
                        rstd, mv[:, 1:2], Act.Sqrt, bias=eps_t, scale=1.0
                    )
                    nc.vector.reciprocal(rstd, rstd)
                    y = yp.tile([128, O], bf16, name="y", tag="yp")
                    nc.vector.tensor_scalar(
                        y, ps, mv[:, 0:1], rstd, Alu.subtract, Alu.mult
                    )
                    for j in range(NOT):
                        pt = psB.tile([128, 128], bf16, name="pt", tag="psB")
                        nc.tensor.transpose(pt, y[:, j * 128 : (j + 1) * 128], ident)
                        nc.scalar.activation(
                            dst_fn(j, m),
                            pt,
                            Act.Identity,
                            bias=bet[j],
                            scale=gam[j],
                        )

                # ---- support projections: K (LN'd, feature-major) + V ------
                for ms in range(NST):
                    xt = sp.tile([128, D], bf16, name="xt", tag="sp")
                    nc.sync.dma_start(out=xt, in_=sTp[ms])
                    ps_k, ps_v = proj_pair(xt, O, 2 * O)
                    ln_transpose(
                        ps_k,
                        lambda j, m: kT[:, j * NS + m * 128 : j * NS + (m + 1) * 128],
                        ms,
                    )
                    nc.vector.tensor_copy(
                        v_all[:, ms * O : (ms + 1) * O], ps_v
                    )

                # ---- query projections ----
                for mq in range(NMT):
                    xt = sp.tile([128, D], bf16, name="xt", tag="sp")
                    nc.sync.dma_start(out=xt, in_=qTp[mq])
                    ps_q, ps_pv = proj_pair(xt, 0, 2 * O)
                    ln_transpose(
                        ps_q,
                        lambda j, m: qqT[j][:, m * 128 : (m + 1) * 128],
                        mq,
                    )
                    qp_sb = ocp.tile([128, O], f32, name="qp_sb", tag="ocp")
                    nc.vector.tensor_copy(qp_sb, ps_pv)
                    nc.sync.dma_start(
                        out=out_q[mq * 128 : (mq + 1) * 128, :], in_=qp_sb
                    )

                # ---- attention: 2 query halves, K/V resident in SBUF -------
                for qh in range(2):
                    sums_ps = psS.tile([128, 4], f32, name="sums_ps", tag="psS")
                    av_ps = [
                        psA.tile([128, O], f32, name=f"av{qi}", tag="psA")
                        for qi in range(4)
                    ]
                    for t in range(n_st):
                        sc = psB.tile([128, O], f32, name="sc", tag="psB")
                        for j in range(NOT):
                            nc.tensor.matmul(
                                sc,
                                kT[:, j * NS + t * 128 : j * NS + (t + 1) * 128],
                                qqT[j][:, qh * O : (qh + 1) * O],
                                start=(j == 0),
                                stop=(j == NOT - 1),
                            )
                        sch = scp.tile([128, O], f16, name="sch", tag="scp")
                        nc.vector.tensor_copy(sch, sc)
                        ex = exl.tile([128, O], bf16, name="ex", tag="exl")
                        nc.scalar.activation(ex, sch, Act.Exp, scale=SCALE)
                        vsl = v_all[:, t * O : (t + 1) * O]
                        for qi in range(4):
                            exq = ex[:, qi * 128 : (qi + 1) * 128]
                            nc.tensor.matmul(
                                av_ps[qi], exq, vsl,
                                start=(t == 0), stop=(t == n_st - 1),
                            )
                            nc.tensor.matmul(
                                sums_ps[:, qi : qi + 1], exq, ones,
                                start=(t == 0), stop=(t == n_st - 1),
                            )
                    rec = stp.tile([128, 4], f32, name="rec", tag="stp")
                    nc.vector.reciprocal(rec, sums_ps)
                    for qi in range(4):
                        oc = ocp.tile([128, O], f32, name="oc", tag="ocp")
                        nc.vector.tensor_scalar_mul(
                            oc, av_ps[qi], rec[:, qi : qi + 1]
                        )
                        row = (qh * 4 + qi) * 128
                        nc.sync.dma_start(out=out_c[row : row + 128, :], in_=oc)

    nc.compile()
    return nc


def _pack_fm(xT):
    # [D, N] feature-major -> [N/128, 128, D]: block (m, p, k*128+b) =
    # xT[k*128+p, m*128+b], so each SBUF load is one contiguous 2-D DMA and
    # xt[:, k*128:(k+1)*128] is the [d, tok] lhsT block for contraction tile k
    n = xT.shape[1]
    return np.ascontiguousarray(
        xT.reshape(NDT, 128, n // 128, 128).transpose(2, 1, 0, 3).reshape(n // 128, 128, D)
    )


def _prep_inputs(support_set, queries, Wq, bq, Wk, bk, Wv, bv, ln_gamma, ln_beta):
    sT = np.ascontiguousarray(np.asarray(support_set, np.float32).T).astype(BF16)
    qT = np.ascontiguousarray(np.asarray(queries, np.float32).T).astype(BF16)
    sTp = _pack_fm(sT)
    w_cat = np.ascontiguousarray(
        np.concatenate(
            [np.asarray(Wq).T, np.asarray(Wk).T, np.asarray(Wv).T], axis=1
        ).astype(np.float32)
    ).astype(BF16)
    brow = np.concatenate(
        [np.asarray(bq), np.asarray(bk), np.asarray(bv)]
    ).astype(np.float32).reshape(1, 3 * O).astype(BF16)

    shared = {
        "sTp": sTp,
        "w": w_cat,
        "brow": np.ascontiguousarray(brow),
        "g_p": np.asarray(ln_gamma, np.float32).reshape(O, 1).copy(),
        "be_p": np.asarray(ln_beta, np.float32).reshape(O, 1).copy(),
    }
    in_maps = []
    for i in range(NCORES):
        m = dict(shared)
        m["qTp"] = _pack_fm(np.ascontiguousarray(qT[:, i * NL : (i + 1) * NL]))
        in_maps.append(m)
    return in_maps


def kernel(support_set, queries, Wq, bq, Wk, bk, Wv, bv, ln_gamma, ln_beta):
    global LAST_RESULTS
    from concourse.bass_utils import run_bass_kernel_spmd

    if "nc" not in _CACHE:
        _CACHE["nc"] = _build_graph()
    nc = _CACHE["nc"]

    in_maps = _prep_inputs(
        support_set, queries, Wq, bq, Wk, bk, Wv, bv, ln_gamma, ln_beta
    )
    _CACHE["in_maps"] = in_maps
    res = run_bass_kernel_spmd(
        nc, in_maps, core_ids=list(range(NCORES)), trace=False
    )
    LAST_RESULTS = res
    q_proto = np.concatenate([res.results[i]["out_q"] for i in range(NCORES)], axis=0)
    c_proto = np.concatenate([res.results[i]["out_c"] for i in range(NCORES)], axis=0)
    return (
        np.asarray(q_proto, np.float32),
        np.asarray(c_proto, np.float32),
    )


def _bench_callable(nc):
    """Single-bind jitted callable over 8 cores with device-resident inputs."""
    import jax
    from jax.experimental.shard_map import shard_map
    from jax.sharding import Mesh, NamedSharding, PartitionSpec

    from concourse import bass2jax, mybir

    in_maps = _CACHE["in_maps"]

    partition_name = (
        nc.partition_id_tensor.name if nc.partition_id_tensor else None
    )
    in_names: list[str] = []
    out_names: list[str] = []
    out_avals = []
    zero_outs = []
    for alloc in nc.m.functions[0].allocations:
        if not isinstance(alloc, mybir.MemoryLocationSet):
            continue
        name = alloc.memorylocations[0].name
        if alloc.kind == "ExternalInput":
            if name != partition_name:
                in_names.append(name)
        elif alloc.kind == "ExternalOutput":
            shape = tuple(alloc.tensor_shape)
            dtype = mybir.dt.np(alloc.dtype)
            out_names.append(name)
            out_avals.append(jax.core.ShapedArray(shape, dtype))
            zero_outs.append(np.zeros(shape, dtype))
    n_params = len(in_names)
    in_names_full = list(in_names) + out_names
    if partition_name is not None:
        in_names_full.append(partition_name)

    def _body(*args):
        operands = list(args)
        if partition_name is not None:
            operands.append(bass2jax.partition_id_tensor())
        outs = bass2jax._bass_exec_p.bind(
            *operands,
            out_avals=tuple(out_avals),
            in_names=tuple(in_names_full),
            out_names=tuple(out_names),
            lowering_input_output_aliases=(),
            sim_require_finite=True,
            sim_require_nnan=True,
            nc=nc,
        )
        return tuple(outs)

    devices = jax.devices()[:NCORES]
    mesh = Mesh(np.asarray(devices), ("core",))
    n_outs = len(out_avals)
    in_specs = (PartitionSpec("core"),) * (n_params + n_outs)
    out_specs = (PartitionSpec("core"),) * n_outs
    sharded = jax.jit(
        shard_map(
            _body, mesh=mesh, in_specs=in_specs, out_specs=out_specs,
            check_rep=False,
        )
    )
    per_core = [
        [np.asarray(in_maps[c][name]) for name in in_names] for c in range(NCORES)
    ]
    concat_in = [
        np.concatenate([per_core[c][i] for c in range(NCORES)], axis=0)
        for i in range(n_params)
    ]
    concat_zeros = [
        np.zeros((NCORES * z.shape[0], *z.shape[1:]), z.dtype) for z in zero_outs
    ]
    sh = NamedSharding(mesh, PartitionSpec("core"))
    dev_in = [jax.device_put(a, sh) for a in concat_in]
    dev_zeros = [jax.device_put(a, sh) for a in concat_zeros]
    jax.block_until_ready(dev_in)
    jax.block_until_ready(dev_zeros)

    def run():
        out = sharded(*dev_in, *dev_zeros)
        jax.block_until_ready(out)
        return out

    return run


def benchmark(n_reps=5, timing_reps=8, **graph_kw):
    """Estimate per-execution device time (ns) by unrolling the kernel body
    n_reps times inside one NEFF and differencing against the 1-rep NEFF."""
    import time

    assert "in_maps" in _CACHE, "call kernel() first"
    key1 = ("bnc", 1, tuple(sorted(graph_kw.items())))
    keyN = ("bnc", n_reps, tuple(sorted(graph_kw.items())))
    if key1 not in _CACHE:
        _CACHE[key1] = _build_graph(reps=1, **graph_kw)
    if keyN not in _CACHE:
        _CACHE[keyN] = _build_graph(reps=n_reps, **graph_kw)
    run1 = _bench_callable(_CACHE[key1])
    runN = _bench_callable(_CACHE[keyN])
    run1()
    runN()  # warm compiles

    def best(fn, k):
        ts = []
        for _ in range(k):
            t0 = time.perf_counter()
            fn()
            ts.append(time.perf_counter() - t0)
        return float(np.min(ts))

    t1 = best(run1, timing_reps)
    tN = best(runN, timing_reps)
    per_exec_s = (tN - t1) / (n_reps - 1)
    return per_exec_s * 1e9, t1, tN
